# revision 1
# baseline (speedup 1.0000x reference)
"""Multi-head attention (B=2, S=2048, D=1024, H=16, d_k=64) on 8 trn2 cores.

Sharding: batch (2) x head-groups (4 groups of 4 heads). Each core computes
its batch's full sequence for its 4 heads plus the partial output projection
(w_o row-sharded); host sums the 4 partials per batch and adds b_o.

Numerics: fp32 PE matmuls on trn2 lower to 2-pass LOW_HIGH at half clock
(~4x slower than bf16). Projections and scores therefore use split-bf16
(x = hi + lo, 3-term hi*hi + hi*lo + lo*hi, fp32 PSUM accumulation,
~2^-18 per-product error). The attention*V matmul, softmax denominators
(a ones-column in the stationary), normalization, and output projection
stay fp32, keeping end-to-end error at fp32 grade (~1e-5).

Layout: all inputs host-pre-transposed to [d_model, seq] so every matmul
contracts along partitions:
  qT/kT  = w_c @ x.T  -> [256, 2048] (hi/lo bf16)
  vh     = x @ w_c.T  -> [2048, 256] fp32 natural (+ ones column)
  scoresT[t, s] (K=64, head pairs packed via tile_position row groups)
  attnT  = exp(scoresT/8)  (no max subtraction: scores ~ N(0,1))
  av     = [vh_h | 1].T @ attnT   (fp32, M=65: row 64 = denominators)
  out   += (av * bcast(1/denom)).T @ w_oT   (fp32 partial)
"""

import numpy as np

P = 128
S = 2048
DM = 1024
DH = 256          # head dims per core (4 heads x 64)
H = 4             # heads per core
DK = 64
MC = DM // P      # 8 m-chunks
TC = S // P       # 16 t-chunks
ST = 1024         # s-tile for scores/exp psum tiles
NST = S // ST     # 2
N_CORES = 8

_COMPILED = None


def _build():
    import concourse.bacc as bacc
    import concourse.mybir as mybir
    from concourse.tile import TileContext

    F32 = mybir.dt.float32
    BF16 = mybir.dt.bfloat16
    AF = mybir.ActivationFunctionType
    OP = mybir.AluOpType

    nc = bacc.Bacc(None, target_bir_lowering=False)

    xin = {}
    win = {}
    for t in ("q", "k", "v"):
        for p in ("h", "l"):
            xin[t + p] = nc.dram_tensor(f"x{t}{p}", [DM, S], BF16, kind="ExternalInput")
            win[t + p] = nc.dram_tensor(f"w{t}{p}", [DM, DH], BF16, kind="ExternalInput")
    bq = nc.dram_tensor("bq", [DH], F32, kind="ExternalInput")
    bk = nc.dram_tensor("bk", [DH], F32, kind="ExternalInput")
    bv = nc.dram_tensor("bv", [DH], F32, kind="ExternalInput")
    woh = nc.dram_tensor("woh", [DH, DM], BF16, kind="ExternalInput")
    wol = nc.dram_tensor("wol", [DH, DM], BF16, kind="ExternalInput")
    out = nc.dram_tensor("out", [S, DM], F32, kind="ExternalOutput")

    with TileContext(nc) as tc:
        with (
            tc.tile_pool(name="persist", bufs=1) as pp,
            tc.tile_pool(name="xfull", bufs=20) as xp,
            tc.tile_pool(name="wstream", bufs=17) as wp,
            tc.tile_pool(name="wstreamv", bufs=9) as wpv,
            tc.tile_pool(name="athl", bufs=8) as hp,
            tc.tile_pool(name="dram", bufs=4, space="DRAM") as dp,
            tc.tile_pool(name="ps_sc", bufs=2, space="PSUM") as ps_sc,
            tc.tile_pool(name="ps_av", bufs=2, space="PSUM") as ps_av,
        ):
            qTh = pp.tile([P, 2, S], BF16, name="qTh")
            qTl = pp.tile([P, 2, S], BF16, name="qTl")
            kTh = pp.tile([P, 2, S], BF16, name="kTh")
            kTl = pp.tile([P, 2, S], BF16, name="kTl")
            vh_h = pp.tile([P, TC, H, DK + 1], BF16, name="vh_h")
            vh_l = pp.tile([P, TC, H, DK + 1], BF16, name="vh_l")
            woh_sb = pp.tile([P, 2, DM], BF16, name="woh_sb")
            wol_sb = pp.tile([P, 2, DM], BF16, name="wol_sb")
            o2ah = pp.tile([P, S], BF16, name="o2ah")  # heads 0,1 normalized hi
            o2al = pp.tile([P, S], BF16, name="o2al")
            o2bh = pp.tile([P, S], BF16, name="o2bh")  # heads 2,3
            o2bl = pp.tile([P, S], BF16, name="o2bl")
            ones = pp.tile([P, DK], F32, name="ones")
            bq_sb = pp.tile([P, 2], F32, name="bq_sb")
            bk_sb = pp.tile([P, 2], F32, name="bk_sb")
            bv_bc = pp.tile([P, DH], F32, name="bv_bc")

            nc.vector.memset(ones[:], 1.0)
            nc.vector.memset(vh_h[:, :, :, DK : DK + 1], 1.0)
            nc.vector.memset(vh_l[:, :, :, DK : DK + 1], 0.0)
            nc.scalar.dma_start(bq_sb[:], bq[:].rearrange("(c p) -> p c", p=P))
            nc.scalar.dma_start(bk_sb[:], bk[:].rearrange("(c p) -> p c", p=P))
            nc.scalar.dma_start(bv_bc[:], bv[None, :].to_broadcast((P, DH)))

            # ---------------- Phase A: projections (split-bf16) -------------
            def load_chunks(xd_h, xd_l, wd_h, wd_l):
                xs, ws = [], []
                for mc in range(MC):
                    xh = xp.tile([P, S], BF16, name="xc")
                    xl = xp.tile([P, S], BF16, name="xc")
                    nc.sync.dma_start(xh[:], xd_h[mc * P : (mc + 1) * P, :])
                    nc.sync.dma_start(xl[:], xd_l[mc * P : (mc + 1) * P, :])
                    wh = wp.tile([P, DH], BF16, name="wc")
                    wl = wp.tile([P, DH], BF16, name="wc")
                    nc.scalar.dma_start(wh[:], wd_h[mc * P : (mc + 1) * P, :])
                    nc.scalar.dma_start(wl[:], wd_l[mc * P : (mc + 1) * P, :])
                    xs.append((xh, xl))
                    ws.append((wh, wl))
                return xs, ws

            # kT / qT: [d' on partitions, s free]; mc-outer streaming so the
            # PE starts as soon as the first chunk lands (4 live psum tiles).
            for t, b_sb, dTh, dTl in (("k", bk_sb, kTh, kTl), ("q", bq_sb, qTh, qTl)):
                xs, ws = load_chunks(xin[t + "h"], xin[t + "l"], win[t + "h"], win[t + "l"])
                tiles = {}
                for dc in range(2):
                    for st2 in range(2):
                        pool = (ps_sc, ps_av)[st2]
                        tiles[(dc, st2)] = pool.tile(
                            [P, ST], F32, name=("sc", "av")[st2]
                        )
                for mc in range(MC):
                    xh, xl = xs[mc]
                    wh, wl = ws[mc]
                    for dc in range(2):
                        for st2 in range(2):
                            for hf in range(2):
                                for ti, (lt, rt) in enumerate(
                                    ((wh, xh), (wh, xl), (wl, xh))
                                ):
                                    nc.tensor.matmul(
                                        tiles[(dc, st2)][:, hf * 512 : (hf + 1) * 512],
                                        lt[:, dc * P : (dc + 1) * P],
                                        rt[:, st2 * ST + hf * 512 : st2 * ST + (hf + 1) * 512],
                                        start=(mc == 0 and ti == 0),
                                        stop=(mc == MC - 1 and ti == 2),
                                    )
                for dc in range(2):
                    for st2 in range(2):
                        ps = tiles[(dc, st2)]
                        sl = (slice(None), dc, slice(st2 * ST, (st2 + 1) * ST))
                        nc.vector.tensor_scalar(
                            out=dTh[sl], in0=ps[:], scalar1=b_sb[:, dc : dc + 1],
                            scalar2=None, op0=OP.add,
                        )
                        nc.vector.scalar_tensor_tensor(
                            out=dTl[sl], in0=ps[:], scalar=b_sb[:, dc : dc + 1],
                            in1=dTh[sl], op0=OP.add, op1=OP.subtract,
                        )

            # vh: natural [t, d'] fp32, + bias broadcast. The wh|wl halves
            # are concatenated on the moving side so xh needs one N=512
            # matmul for two split terms; the xl*wh term rides N=256.
            xsv, wsv = [], []
            for mc in range(MC):
                xh = xp.tile([P, S], BF16, name="xc")
                xl = xp.tile([P, S], BF16, name="xc")
                nc.sync.dma_start(xh[:], xin["vh"][mc * P : (mc + 1) * P, :])
                nc.sync.dma_start(xl[:], xin["vl"][mc * P : (mc + 1) * P, :])
                whl = wpv.tile([P, 2, DH], BF16, name="wcv")
                nc.scalar.dma_start(whl[:, 0, :], win["vh"][mc * P : (mc + 1) * P, :])
                nc.scalar.dma_start(whl[:, 1, :], win["vl"][mc * P : (mc + 1) * P, :])
                xsv.append((xh, xl))
                wsv.append(whl)
            for tcc in range(TC):
                pool = (ps_sc, ps_av)[tcc % 2]
                ps = pool.tile([P, 512], F32, name=("sc", "av")[tcc % 2])
                for mc in range(MC):
                    xh, xl = xsv[mc]
                    whl = wsv[mc]
                    nc.tensor.matmul(
                        ps[:, 0:512],
                        xh[:, tcc * P : (tcc + 1) * P],
                        whl[:].rearrange("p a b -> p (a b)"),
                        start=(mc == 0),
                        stop=False,
                    )
                    nc.tensor.matmul(
                        ps[:, 0:DH],
                        xl[:, tcc * P : (tcc + 1) * P],
                        whl[:, 0, :],
                        start=False,
                        stop=(mc == MC - 1),
                    )
                tmpv = xp.tile([P, DH], F32, name="xc")
                nc.vector.tensor_tensor(
                    out=tmpv[:], in0=ps[:, DH:512], in1=bv_bc[:], op=OP.add
                )
                vhf = xp.tile([P, DH], F32, name="xc")
                nc.vector.tensor_tensor(
                    out=vhf[:], in0=ps[:, 0:DH], in1=tmpv[:], op=OP.add
                )
                hsl = vh_h[:, tcc, :, 0:DK]
                lsl = vh_l[:, tcc, :, 0:DK]
                vhf3 = vhf[:].rearrange("p (h d) -> p h d", h=H)
                nc.vector.tensor_copy(hsl, vhf3)
                nc.vector.tensor_tensor(out=lsl, in0=vhf3, in1=hsl, op=OP.subtract)

            # ---------------- Phase B: attention ----------------
            # normalize part 2 of the previous block is deferred into the next
            # block's score stream so the PE never idles at block boundaries.
            pending_norm2 = []
            for pair in range(2):
                o2h, o2l = ((o2ah, o2al), (o2bh, o2bl))[pair]
                for st2 in range(NST):
                    s0 = st2 * ST
                    avs = [ps_av.tile([P, ST], F32, name="av") for _ in range(2)]
                    ats = {}

                    def scores_exp(tcc):
                        for hi2 in range(2):
                            rows = slice(DK * hi2, DK * (hi2 + 1))
                            sc = ps_sc.tile([P, ST], F32, name="sc")
                            for hf in range(2):
                                i = 0
                                for lt, rt in ((kTh, qTh), (kTh, qTl), (kTl, qTh)):
                                    nc.tensor.matmul(
                                        sc[:, hf * 512 : (hf + 1) * 512],
                                        lt[rows, pair, tcc * P : (tcc + 1) * P],
                                        rt[rows, pair, s0 + hf * 512 : s0 + (hf + 1) * 512],
                                        start=(i == 0),
                                        stop=(i == 2),
                                        tile_position=(DK * hi2, 0),
                                    )
                                    i += 1
                            at = xp.tile([P, ST], F32, name="xc")
                            nc.scalar.activation(at[:], sc[:], AF.Exp, scale=0.125)
                            ath = hp.tile([P, ST], BF16, name="ath")
                            atl = hp.tile([P, ST], BF16, name="atl")
                            nc.vector.tensor_copy(ath[:], at[:])
                            nc.vector.tensor_tensor(
                                out=atl[:], in0=at[:], in1=ath[:], op=OP.subtract
                            )
                            ats[(tcc, hi2)] = (ath, atl)

                    def av_mm(tcc):
                        for hi2 in range(2):
                            ath, atl = ats.pop((tcc, hi2))
                            h = 2 * pair + hi2
                            for hf in range(2):
                                terms = (
                                    (vh_h, ath), (vh_h, atl), (vh_l, ath)
                                )
                                for ti, (lv, ra) in enumerate(terms):
                                    nc.tensor.matmul(
                                        avs[hi2][0 : DK + 1, hf * 512 : (hf + 1) * 512],
                                        lv[:, tcc, h, :],
                                        ra[:, hf * 512 : (hf + 1) * 512],
                                        start=(tcc == 0 and ti == 0),
                                        stop=(tcc == TC - 1 and ti == 2),
                                    )

                    # software pipeline: scores run ~2 ahead of av; the
                    # previous block's deferred normalize ops are spread one
                    # per av step so they never bunch up in the DVE queue.
                    scores_exp(0)
                    scores_exp(1)
                    for tcc in range(2, TC):
                        scores_exp(tcc)
                        av_mm(tcc - 2)
                        if pending_norm2:
                            pending_norm2.pop(0)()
                    av_mm(TC - 2)
                    av_mm(TC - 1)
                    for n2 in pending_norm2:
                        n2()
                    pending_norm2 = []

                    # normalize part 1: copy unnormalized rows out of PSUM
                    # (to a transient f32 tile at the matching partition base)
                    # and compute the reciprocal of the denominator row.
                    part1 = []
                    for hi2 in range(2):
                        av = avs[hi2]
                        rows = slice(DK * hi2, DK * (hi2 + 1))
                        u = xp.tile([P, ST], F32, name="xc")
                        nc.vector.tensor_copy(u[rows, :], av[0:DK, :])
                        dsb = xp.tile([1, ST], F32, name="xc")
                        nc.scalar.copy(dsb[0:1, :], av[DK : DK + 1, :])
                        part1.append((u, dsb))
                    norm_rs = []
                    for hi2 in range(2):
                        u, dsb = part1[hi2]
                        rsb = xp.tile([1, ST], F32, name="xc")
                        scr = xp.tile([1, ST], F32, name="xc")
                        nc.vector.reciprocal_approx_accurate(
                            rsb[0:1, :], dsb[0:1, :], scr[0:1, :]
                        )
                        rdr = dp.tile([1, ST], F32, name="rdr")
                        nc.sync.dma_start(rdr[0:1, :], rsb[0:1, :])
                        norm_rs.append((rdr, u))

                    # normalize part 2 (closure): K=1 matmul broadcasts the
                    # reciprocal; in-place multiply (SBUF x PSUM).
                    def make_norm2(o2h=o2h, o2l=o2l, s0=s0, norm_rs=norm_rs):
                        thunks = []
                        for hi2 in range(2):
                            rdr, u = norm_rs[hi2]
                            rows = slice(DK * hi2, DK * (hi2 + 1))
                            sl = (rows, slice(s0, s0 + ST))
                            rb = xp.tile([P, ST], F32, name="xc")

                            def t1(rdr=rdr, u=u, rb=rb, rows=rows):
                                nc.sync.dma_start(
                                    rb[rows, :], rdr[0:1, :].to_broadcast((DK, ST))
                                )
                                nc.vector.tensor_mul(u[rows, :], u[rows, :], rb[rows, :])

                            def t2(u=u, rb=rb, rows=rows, sl=sl):
                                nc.vector.tensor_copy(o2h[sl], u[rows, :])

                            def t3(u=u, rows=rows, sl=sl):
                                nc.vector.tensor_tensor(
                                    out=o2l[sl], in0=u[rows, :], in1=o2h[sl],
                                    op=OP.subtract,
                                )

                            thunks += [t1, t2, t3]
                        return thunks

                    pending_norm2.extend(make_norm2())

            # ---------------- Phase C: output projection (split-bf16) -------
            # st7 0..7 read s-columns < 1024, whose normalize (st2=0 blocks)
            # has already drained; emit them before the final norm2 drain so
            # the PE covers the last block's normalize latency.
            nc.scalar.dma_start(woh_sb[:], woh[:].rearrange("(c p) n -> p c n", p=P))
            nc.scalar.dma_start(wol_sb[:], wol[:].rearrange("(c p) n -> p c n", p=P))
            st7_order = list(range(TC // 2)) + [-1] + list(range(TC // 2, TC))
            for st7 in st7_order:
                if st7 == -1:
                    for n2 in pending_norm2:
                        n2()
                    pending_norm2 = []
                    continue
                if st7 < TC // 2 and pending_norm2:
                    pending_norm2.pop(0)()
                of_ps = ps_av.tile([P, ST], F32, name="av")
                for c in range(2):
                    o2h, o2l = ((o2ah, o2al), (o2bh, o2bl))[c]
                    for nh in range(2):
                        terms = (
                            (o2h, woh_sb), (o2h, wol_sb), (o2l, woh_sb)
                        )
                        for ti, (lo2, rwo) in enumerate(terms):
                            nc.tensor.matmul(
                                of_ps[:, nh * 512 : (nh + 1) * 512],
                                lo2[:, st7 * P : (st7 + 1) * P],
                                rwo[:, c, nh * 512 : (nh + 1) * 512],
                                start=(c == 0 and ti == 0),
                                stop=(c == 1 and ti == 2),
                            )
                of = xp.tile([P, ST], F32, name="xc")
                nc.vector.tensor_copy(of[:], of_ps[:])
                nc.sync.dma_start(out[st7 * P : (st7 + 1) * P, :], of[:])

    nc.compile()
    return nc


def _get_nc():
    global _COMPILED
    if _COMPILED is None:
        _COMPILED = _build()
    return _COMPILED


def _split_bf16(x):
    import ml_dtypes

    hi = np.ascontiguousarray(x.astype(ml_dtypes.bfloat16))
    lo = np.ascontiguousarray(
        (x - hi.astype(np.float32)).astype(ml_dtypes.bfloat16)
    )
    return hi, lo


def _make_in_maps(q, k, v, w_q, b_q, w_k, b_k, w_v, b_v, w_o, b_o):
    q = np.asarray(q, np.float32)
    k = np.asarray(k, np.float32)
    v = np.asarray(v, np.float32)
    xs = {}
    for t, arr in (("q", q), ("k", k), ("v", v)):
        for b in range(2):
            xs[(t, b)] = _split_bf16(np.ascontiguousarray(arr[b].T))
    ws = {"q": np.asarray(w_q, np.float32), "k": np.asarray(w_k, np.float32),
          "v": np.asarray(w_v, np.float32)}
    bs = {"q": np.asarray(b_q, np.float32), "k": np.asarray(b_k, np.float32),
          "v": np.asarray(b_v, np.float32)}
    w_o = np.asarray(w_o, np.float32)
    in_maps = []
    for core in range(N_CORES):
        b, hg = divmod(core, 4)
        sl = slice(hg * DH, (hg + 1) * DH)
        m = {}
        for t in ("q", "k", "v"):
            m[f"x{t}h"], m[f"x{t}l"] = xs[(t, b)]
            wh, wl = _split_bf16(np.ascontiguousarray(ws[t][sl, :].T))
            m[f"w{t}h"], m[f"w{t}l"] = wh, wl
            m[f"b{t}"] = np.ascontiguousarray(bs[t][sl])
        m["woh"], m["wol"] = _split_bf16(np.ascontiguousarray(w_o[:, sl].T))
        in_maps.append(m)
    return in_maps


def run(inputs, trace=False):
    from concourse.bass_utils import run_bass_kernel_spmd

    nc = _get_nc()
    in_maps = _make_in_maps(**inputs)
    res = run_bass_kernel_spmd(
        nc, in_maps, core_ids=list(range(N_CORES)), trace=trace
    )
    b_o = np.asarray(inputs["b_o"], np.float32)
    full = np.empty((2, S, DM), np.float32)
    for b in range(2):
        acc = res.results[4 * b]["out"].astype(np.float32)
        for hg in range(1, 4):
            acc = acc + res.results[4 * b + hg]["out"]
        full[b] = acc + b_o[None, :]
    return full, res


def kernel(**inputs) -> np.ndarray:
    full, _ = run(inputs, trace=False)
    return full



# revision 2
# speedup vs baseline: 1.7866x; 1.7866x over previous
"""Multi-head attention (B=2, S=2048, D=1024, H=16, d_k=64) on 8 trn2 cores.

Sharding: batch (2) x head-groups (4 groups of 4 heads). Each core computes
its batch's full sequence for its 4 heads plus the partial output projection
(w_o row-sharded); host sums the 4 partials per batch and adds b_o.

Numerics: single-pass bf16 matmuls with fp32 PSUM accumulation everywhere
(the 2e-2 rel-err budget has ~10x headroom over bf16-grade ~2e-3 error).
exp() writes bf16 directly so the attention probabilities feed the AV
matmul with no intermediate cast.

Layout: all inputs host-pre-transposed to [d_model, seq]:
  qT/kT  = w_c @ x.T  -> [128(d' pair), 2, 2048] bf16
  vh     = x @ w_c.T  -> [128(t), tc, h, 65] bf16 (+ ones column)
  scoresT[t, s] (K=64, head pairs packed via tile_position row groups)
  atT    = exp(scoresT/8) bf16  (no max subtraction: scores ~ N(0,1))
  av     = [vh_h | 1].T @ atT   (fp32 psum, M=65: row 64 = denominators)
  out   += (av * bcast(1/denom)).T @ w_oT   (fp32 partial)
"""

import numpy as np

P = 128
S = 2048
DM = 1024
DH = 256          # head dims per core (4 heads x 64)
H = 4             # heads per core
DK = 64
MC = DM // P      # 8 m-chunks
TC = S // P       # 16 t-chunks
ST = 1024         # s-tile for scores/exp psum tiles
NST = S // ST     # 2
N_CORES = 8

_COMPILED = None


def _build():
    import concourse.bacc as bacc
    import concourse.mybir as mybir
    from concourse.tile import TileContext

    F32 = mybir.dt.float32
    BF16 = mybir.dt.bfloat16
    AF = mybir.ActivationFunctionType
    OP = mybir.AluOpType

    nc = bacc.Bacc(None, target_bir_lowering=False)

    xin = {}
    win = {}
    for t in ("q", "k", "v"):
        xin[t] = nc.dram_tensor(f"x{t}", [DM, S], BF16, kind="ExternalInput")
        win[t] = nc.dram_tensor(f"w{t}", [DM, DH], BF16, kind="ExternalInput")
    bq = nc.dram_tensor("bq", [DH], F32, kind="ExternalInput")
    bk = nc.dram_tensor("bk", [DH], F32, kind="ExternalInput")
    bv = nc.dram_tensor("bv", [DH], F32, kind="ExternalInput")
    wo = nc.dram_tensor("wo", [DH, DM], BF16, kind="ExternalInput")
    out = nc.dram_tensor("out", [S, DM], F32, kind="ExternalOutput")

    with TileContext(nc) as tc:
        with (
            tc.tile_pool(name="persist", bufs=1) as pp,
            tc.tile_pool(name="xfull", bufs=24) as xw,
            tc.tile_pool(name="wstream", bufs=24) as wp,
            tc.tile_pool(name="trans", bufs=10) as xp,
            tc.tile_pool(name="athl", bufs=8) as hp,
            tc.tile_pool(name="dram", bufs=4, space="DRAM") as dp,
            tc.tile_pool(name="ps_sc", bufs=2, space="PSUM") as ps_sc,
            tc.tile_pool(name="ps_av", bufs=2, space="PSUM") as ps_av,
        ):
            qT = pp.tile([P, 2, S], BF16, name="qT")
            kT = pp.tile([P, 2, S], BF16, name="kT")
            vh = pp.tile([P, TC, H, DK + 1], BF16, name="vh")
            wo_sb = pp.tile([P, 2, DM], BF16, name="wo_sb")
            o2a = pp.tile([P, S], BF16, name="o2a")  # heads 0,1 normalized
            o2b = pp.tile([P, S], BF16, name="o2b")  # heads 2,3
            bq_sb = pp.tile([P, 2], F32, name="bq_sb")
            bk_sb = pp.tile([P, 2], F32, name="bk_sb")
            bv_bc = pp.tile([P, DH], F32, name="bv_bc")

            nc.vector.memset(vh[:, :, :, DK : DK + 1], 1.0)
            nc.scalar.dma_start(bq_sb[:], bq[:].rearrange("(c p) -> p c", p=P))
            nc.scalar.dma_start(bk_sb[:], bk[:].rearrange("(c p) -> p c", p=P))
            nc.scalar.dma_start(bv_bc[:], bv[None, :].to_broadcast((P, DH)))

            # ---------------- Phase A: projections (bf16) -------------------
            xs = {}
            ws = {}
            for t in ("k", "q", "v"):
                for mc in range(MC):
                    xt = xw.tile([P, S], BF16, name="xc")
                    nc.sync.dma_start(xt[:], xin[t][mc * P : (mc + 1) * P, :])
                    wt = wp.tile([P, DH], BF16, name="wc")
                    nc.scalar.dma_start(wt[:], win[t][mc * P : (mc + 1) * P, :])
                    xs[(t, mc)] = xt
                    ws[(t, mc)] = wt

            # kT / qT: [d' on partitions, s free]; mc-outer streaming so the
            # PE starts as soon as the first chunk lands.
            for t, b_sb, dT in (("k", bk_sb, kT), ("q", bq_sb, qT)):
                tiles = {}
                for dc in range(2):
                    for st2 in range(2):
                        pool = (ps_sc, ps_av)[st2]
                        tiles[(dc, st2)] = pool.tile(
                            [P, ST], F32, name=("sc", "av")[st2]
                        )
                for mc in range(MC):
                    for dc in range(2):
                        for st2 in range(2):
                            for hf in range(2):
                                nc.tensor.matmul(
                                    tiles[(dc, st2)][:, hf * 512 : (hf + 1) * 512],
                                    ws[(t, mc)][:, dc * P : (dc + 1) * P],
                                    xs[(t, mc)][
                                        :, st2 * ST + hf * 512 : st2 * ST + (hf + 1) * 512
                                    ],
                                    start=(mc == 0),
                                    stop=(mc == MC - 1),
                                )
                for dc in range(2):
                    for st2 in range(2):
                        ps = tiles[(dc, st2)]
                        sl = (slice(None), dc, slice(st2 * ST, (st2 + 1) * ST))
                        nc.vector.tensor_scalar(
                            out=dT[sl], in0=ps[:], scalar1=b_sb[:, dc : dc + 1],
                            scalar2=None, op0=OP.add,
                        )

            # vh: natural [t, d'] fp32 psum + bias broadcast -> bf16.
            for tcc in range(TC):
                pool = (ps_sc, ps_av)[tcc % 2]
                ps = pool.tile([P, DH], F32, name=("sc", "av")[tcc % 2])
                for mc in range(MC):
                    nc.tensor.matmul(
                        ps[:],
                        xs[("v", mc)][:, tcc * P : (tcc + 1) * P],
                        ws[("v", mc)][:],
                        start=(mc == 0),
                        stop=(mc == MC - 1),
                    )
                nc.vector.tensor_tensor(
                    out=vh[:, tcc, :, 0:DK],
                    in0=ps[:].rearrange("p (h d) -> p h d", h=H),
                    in1=bv_bc[:].rearrange("p (h d) -> p h d", h=H),
                    op=OP.add,
                )

            # ---------------- Phase B: attention ----------------
            # normalize part 2 of the previous block is deferred into the next
            # block's score stream so the PE never idles at block boundaries.
            pending_norm2 = []
            for pair in range(2):
                o2h = (o2a, o2b)[pair]
                for st2 in range(NST):
                    s0 = st2 * ST
                    avs = [ps_av.tile([P, ST], F32, name="av") for _ in range(2)]
                    ats = {}

                    def scores_exp(tcc):
                        for hi2 in range(2):
                            rows = slice(DK * hi2, DK * (hi2 + 1))
                            sc = ps_sc.tile([P, ST], F32, name="sc")
                            for hf in range(2):
                                nc.tensor.matmul(
                                    sc[:, hf * 512 : (hf + 1) * 512],
                                    kT[rows, pair, tcc * P : (tcc + 1) * P],
                                    qT[rows, pair, s0 + hf * 512 : s0 + (hf + 1) * 512],
                                    start=True,
                                    stop=True,
                                    tile_position=(DK * hi2, 0),
                                )
                            at = hp.tile([P, ST], BF16, name="at")
                            nc.scalar.activation(at[:], sc[:], AF.Exp, scale=0.125)
                            ats[(tcc, hi2)] = at

                    def av_mm(tcc):
                        for hi2 in range(2):
                            at = ats.pop((tcc, hi2))
                            h = 2 * pair + hi2
                            for hf in range(2):
                                nc.tensor.matmul(
                                    avs[hi2][0 : DK + 1, hf * 512 : (hf + 1) * 512],
                                    vh[:, tcc, h, :],
                                    at[:, hf * 512 : (hf + 1) * 512],
                                    start=(tcc == 0),
                                    stop=(tcc == TC - 1),
                                )

                    # software pipeline: scores run ~2 ahead of av; the
                    # previous block's deferred normalize ops are spread one
                    # per step so they never bunch up in the DVE queue.
                    scores_exp(0)
                    scores_exp(1)
                    for tcc in range(2, TC):
                        scores_exp(tcc)
                        av_mm(tcc - 2)
                        if pending_norm2:
                            pending_norm2.pop(0)()
                    av_mm(TC - 2)
                    av_mm(TC - 1)
                    for n2 in pending_norm2:
                        n2()
                    pending_norm2 = []

                    # normalize part 1: copy unnormalized rows out of PSUM
                    # (to a transient f32 tile at the matching partition base)
                    # and compute the reciprocal of the denominator row.
                    norm_rs = []
                    for hi2 in range(2):
                        av = avs[hi2]
                        rows = slice(DK * hi2, DK * (hi2 + 1))
                        u = xp.tile([P, ST], F32, name="xc")
                        nc.vector.tensor_copy(u[rows, :], av[0:DK, :])
                        dsb = xp.tile([1, ST], F32, name="xc")
                        nc.scalar.copy(dsb[0:1, :], av[DK : DK + 1, :])
                        rsb = xp.tile([1, ST], F32, name="xc")
                        scr = xp.tile([1, ST], F32, name="xc")
                        nc.vector.reciprocal_approx_accurate(
                            rsb[0:1, :], dsb[0:1, :], scr[0:1, :]
                        )
                        rdr = dp.tile([1, ST], F32, name="rdr")
                        nc.sync.dma_start(rdr[0:1, :], rsb[0:1, :])
                        norm_rs.append((rdr, u))

                    # normalize part 2 (closure): broadcast the reciprocal
                    # via DRAM round-trip, then fused multiply+cast to bf16.
                    def make_norm2(o2h=o2h, s0=s0, norm_rs=norm_rs):
                        thunks = []
                        for hi2 in range(2):
                            rdr, u = norm_rs[hi2]
                            rows = slice(DK * hi2, DK * (hi2 + 1))
                            sl = (rows, slice(s0, s0 + ST))
                            rb = xp.tile([P, ST], F32, name="xc")

                            def t1(rdr=rdr, rb=rb, rows=rows):
                                nc.sync.dma_start(
                                    rb[rows, :], rdr[0:1, :].to_broadcast((DK, ST))
                                )

                            def t2(u=u, rb=rb, rows=rows, sl=sl, o2h=o2h):
                                nc.vector.tensor_tensor(
                                    out=o2h[sl], in0=u[rows, :], in1=rb[rows, :],
                                    op=OP.mult,
                                )

                            thunks += [t1, t2]
                        return thunks

                    pending_norm2.extend(make_norm2())

            # ---------------- Phase C: output projection --------------------
            # st7 0..7 read s-columns < 1024, whose normalize (st2=0 blocks)
            # has already drained; emit them before the final norm2 drain so
            # the PE covers the last block's normalize latency.
            nc.scalar.dma_start(wo_sb[:], wo[:].rearrange("(c p) n -> p c n", p=P))
            st7_order = list(range(TC // 2)) + [-1] + list(range(TC // 2, TC))
            for st7 in st7_order:
                if st7 == -1:
                    for n2 in pending_norm2:
                        n2()
                    pending_norm2 = []
                    continue
                if st7 < TC // 2 and pending_norm2:
                    pending_norm2.pop(0)()
                of_ps = ps_av.tile([P, ST], F32, name="av")
                for c in range(2):
                    o2h = (o2a, o2b)[c]
                    for nh in range(2):
                        nc.tensor.matmul(
                            of_ps[:, nh * 512 : (nh + 1) * 512],
                            o2h[:, st7 * P : (st7 + 1) * P],
                            wo_sb[:, c, nh * 512 : (nh + 1) * 512],
                            start=(c == 0),
                            stop=(c == 1),
                        )
                of = xp.tile([P, ST], F32, name="xc")
                nc.vector.tensor_copy(of[:], of_ps[:])
                nc.sync.dma_start(out[st7 * P : (st7 + 1) * P, :], of[:])

    nc.compile()
    return nc


def _get_nc():
    global _COMPILED
    if _COMPILED is None:
        _COMPILED = _build()
    return _COMPILED


def _bf16(x):
    import ml_dtypes

    return np.ascontiguousarray(x.astype(ml_dtypes.bfloat16))


def _make_in_maps(q, k, v, w_q, b_q, w_k, b_k, w_v, b_v, w_o, b_o):
    q = np.asarray(q, np.float32)
    k = np.asarray(k, np.float32)
    v = np.asarray(v, np.float32)
    xs = {}
    for t, arr in (("q", q), ("k", k), ("v", v)):
        for b in range(2):
            xs[(t, b)] = _bf16(np.ascontiguousarray(arr[b].T))
    ws = {"q": np.asarray(w_q, np.float32), "k": np.asarray(w_k, np.float32),
          "v": np.asarray(w_v, np.float32)}
    bs = {"q": np.asarray(b_q, np.float32), "k": np.asarray(b_k, np.float32),
          "v": np.asarray(b_v, np.float32)}
    w_o = np.asarray(w_o, np.float32)
    in_maps = []
    for core in range(N_CORES):
        b, hg = divmod(core, 4)
        sl = slice(hg * DH, (hg + 1) * DH)
        m = {}
        for t in ("q", "k", "v"):
            m[f"x{t}"] = xs[(t, b)]
            m[f"w{t}"] = _bf16(np.ascontiguousarray(ws[t][sl, :].T))
            m[f"b{t}"] = np.ascontiguousarray(bs[t][sl])
        m["wo"] = _bf16(np.ascontiguousarray(w_o[:, sl].T))
        in_maps.append(m)
    return in_maps


def run(inputs, trace=False):
    from concourse.bass_utils import run_bass_kernel_spmd

    nc = _get_nc()
    in_maps = _make_in_maps(**inputs)
    res = run_bass_kernel_spmd(
        nc, in_maps, core_ids=list(range(N_CORES)), trace=trace
    )
    b_o = np.asarray(inputs["b_o"], np.float32)
    full = np.empty((2, S, DM), np.float32)
    for b in range(2):
        acc = res.results[4 * b]["out"].astype(np.float32)
        for hg in range(1, 4):
            acc = acc + res.results[4 * b + hg]["out"]
        full[b] = acc + b_o[None, :]
    return full, res


def kernel(**inputs) -> np.ndarray:
    full, _ = run(inputs, trace=False)
    return full


# revision 4
# speedup vs baseline: 1.8383x; 1.0289x over previous
"""Multi-head attention (B=2, S=2048, D=1024, H=16, d_k=64) on 8 trn2 cores.

Sharding: batch (2) x head-groups (4 groups of 4 heads). Each core computes
its batch's full sequence for its 4 heads plus the partial output projection
(w_o row-sharded); host sums the 4 bf16 partials per batch and adds b_o.

Numerics: single-pass bf16 matmuls with fp32 PSUM accumulation everywhere
(the 2e-2 rel-err budget has ~10x headroom over bf16-grade ~2e-3 error).
exp() writes bf16 directly so the attention probabilities feed the AV
matmul with no intermediate cast.

Schedule: the exp stream on the scalar engine is the pacer (~1.11us per
[128,1024] tile, 128 tiles). Projections are issued per head-pair so the
first attention block starts as early as possible; the remaining
projections are issued in the inter-block gaps where the scalar engine
still has exp backlog. Input DMA is split across both HWDGE rings
(sync: x_k/x_q, scalar: weights + x_v + w_o).

Layout: all inputs host-pre-transposed to [d_model, seq]:
  qT/kT  = w_c @ x.T  -> [128(d' pair), 2, 2048] bf16
  vh     = x @ w_c.T  -> [128(t), tc, h, 65] bf16 (+ ones column)
  scoresT[t, s] (K=64, head pairs packed via tile_position row groups)
  atT    = exp(scoresT/8) bf16  (no max subtraction: scores ~ N(0,1))
  av     = [vh_h | 1].T @ atT   (fp32 psum, M=65: row 64 = denominators)
  out   += (av * bcast(1/denom)).T @ w_oT   (bf16 partial)
"""

import numpy as np

P = 128
S = 2048
DM = 1024
DH = 256          # head dims per core (4 heads x 64)
H = 4             # heads per core
DK = 64
MC = DM // P      # 8 m-chunks
TC = S // P       # 16 t-chunks
ST = 1024         # s-tile for scores/exp psum tiles
NST = S // ST     # 2
N_CORES = 8

_COMPILED = None


def _build():
    import concourse.bacc as bacc
    import concourse.mybir as mybir
    from concourse.tile import TileContext

    F32 = mybir.dt.float32
    BF16 = mybir.dt.bfloat16
    AF = mybir.ActivationFunctionType
    OP = mybir.AluOpType

    nc = bacc.Bacc(None, target_bir_lowering=False)

    xin = {}
    win = {}
    for t in ("q", "k", "v"):
        xin[t] = nc.dram_tensor(f"x{t}", [DM, S], BF16, kind="ExternalInput")
        win[t] = nc.dram_tensor(f"w{t}", [DM, DH], BF16, kind="ExternalInput")
    bq = nc.dram_tensor("bq", [DH], F32, kind="ExternalInput")
    bk = nc.dram_tensor("bk", [DH], F32, kind="ExternalInput")
    bv = nc.dram_tensor("bv", [DH], F32, kind="ExternalInput")
    wo = nc.dram_tensor("wo", [DH, DM], BF16, kind="ExternalInput")
    out = nc.dram_tensor("out", [S, DM], BF16, kind="ExternalOutput")

    with TileContext(nc) as tc:
        with (
            tc.tile_pool(name="persist", bufs=1) as pp,
            tc.tile_pool(name="xfull", bufs=6) as xw,
            tc.tile_pool(name="trans", bufs=10) as xp,
            tc.tile_pool(name="athl", bufs=8) as hp,
            tc.tile_pool(name="dram", bufs=4, space="DRAM") as dp,
            tc.tile_pool(name="ps_sc", bufs=2, space="PSUM") as ps_sc,
            tc.tile_pool(name="ps_av", bufs=2, space="PSUM") as ps_av,
        ):
            qT = pp.tile([P, 2, S], BF16, name="qT")
            kT = pp.tile([P, 2, S], BF16, name="kT")
            vh = pp.tile([P, TC, H, DK + 1], BF16, name="vh")
            wo_sb = pp.tile([P, 2, DM], BF16, name="wo_sb")
            o2a = pp.tile([P, S], BF16, name="o2a")  # heads 0,1 normalized
            o2b = pp.tile([P, S], BF16, name="o2b")  # heads 2,3
            bq_sb = pp.tile([P, 2], F32, name="bq_sb")
            bk_sb = pp.tile([P, 2], F32, name="bk_sb")
            bv_bc = pp.tile([P, DH], F32, name="bv_bc")

            nc.vector.memset(vh[:, :, :, DK : DK + 1], 1.0)
            nc.sync.dma_start(bq_sb[:], bq[:].rearrange("(c p) -> p c", p=P))
            nc.sync.dma_start(bk_sb[:], bk[:].rearrange("(c p) -> p c", p=P))
            nc.sync.dma_start(bv_bc[:], bv[None, :].to_broadcast((P, DH)))

            # ---------------- input DMA (two HWDGE rings) -------------------
            # sync ring: x_k then x_q (1 MB transfers). scalar ring: all
            # weights first (small), then x_v, then w_o. The scalar engine
            # only issues these before its first exp, so nothing competes
            # with the activation stream later.
            xt = {}
            for t in ("k", "q", "v"):
                xt[t] = [xw.tile([P, MC // 2, S], BF16, name="xc") for _ in range(2)]
            wt = {}
            for t in ("k", "q", "v"):
                w = pp.tile([P, MC, DH], BF16, name=f"w{t}_sb")
                nc.scalar.dma_start(
                    w[:], win[t][:].rearrange("(c p) n -> p c n", p=P)
                )
                wt[t] = w
            for t in ("k", "q"):
                for h2 in range(2):
                    nc.sync.dma_start(
                        xt[t][h2][:],
                        xin[t][h2 * 512 : (h2 + 1) * 512, :].rearrange(
                            "(c p) s -> p c s", p=P
                        ),
                    )
            for h2 in range(2):
                nc.scalar.dma_start(
                    xt["v"][h2][:],
                    xin["v"][h2 * 512 : (h2 + 1) * 512, :].rearrange(
                        "(c p) s -> p c s", p=P
                    ),
                )
            nc.scalar.dma_start(wo_sb[:], wo[:].rearrange("(c p) n -> p c n", p=P))

            def xsl(t, mc):
                return xt[t][mc // 4][:, mc % 4, :]

            # ---------------- projections, issued per pair ------------------
            def proj_kq(t, b_sb, dT, pair, st2s):
                tiles = {}
                for st2 in st2s:
                    pool = (ps_sc, ps_av)[st2]
                    tiles[st2] = pool.tile([P, ST], F32, name=("sc", "av")[st2])
                for mc in range(MC):
                    for st2 in st2s:
                        for hf in range(2):
                            nc.tensor.matmul(
                                tiles[st2][:, hf * 512 : (hf + 1) * 512],
                                wt[t][:, mc, pair * P : (pair + 1) * P],
                                xsl(t, mc)[
                                    :, st2 * ST + hf * 512 : st2 * ST + (hf + 1) * 512
                                ],
                                start=(mc == 0),
                                stop=(mc == MC - 1),
                            )
                for st2 in st2s:
                    sl = (slice(None), pair, slice(st2 * ST, (st2 + 1) * ST))
                    nc.vector.tensor_scalar(
                        out=dT[sl], in0=tiles[st2][:], scalar1=b_sb[:, pair : pair + 1],
                        scalar2=None, op0=OP.add,
                    )

            def proj_vh(pair):
                for tcc in range(TC):
                    pool = (ps_sc, ps_av)[tcc % 2]
                    ps = pool.tile([P, P], F32, name=("sc", "av")[tcc % 2])
                    for mc in range(MC):
                        nc.tensor.matmul(
                            ps[:],
                            xsl("v", mc)[:, tcc * P : (tcc + 1) * P],
                            wt["v"][:, mc, pair * P : (pair + 1) * P],
                            start=(mc == 0),
                            stop=(mc == MC - 1),
                        )
                    nc.vector.tensor_tensor(
                        out=vh[:, tcc, 2 * pair : 2 * pair + 2, 0:DK],
                        in0=ps[:].rearrange("p (h d) -> p h d", h=2),
                        in1=bv_bc[:, pair * P : (pair + 1) * P].rearrange(
                            "p (h d) -> p h d", h=2
                        ),
                        op=OP.add,
                    )

            # ---------------- attention block -------------------------------
            pending = []  # deferred thunks drained one per tcc step

            def drain_one():
                if pending:
                    pending.pop(0)()

            def attn_block(pair, st2):
                o2h = (o2a, o2b)[pair]
                s0 = st2 * ST
                avs = [ps_av.tile([P, ST], F32, name="av") for _ in range(2)]
                ats = {}

                def scores_exp(tcc):
                    for hi2 in range(2):
                        rows = slice(DK * hi2, DK * (hi2 + 1))
                        sc = ps_sc.tile([P, ST], F32, name="sc")
                        for hf in range(2):
                            nc.tensor.matmul(
                                sc[:, hf * 512 : (hf + 1) * 512],
                                kT[rows, pair, tcc * P : (tcc + 1) * P],
                                qT[rows, pair, s0 + hf * 512 : s0 + (hf + 1) * 512],
                                start=True,
                                stop=True,
                                tile_position=(DK * hi2, 0),
                            )
                        at = hp.tile([P, ST], BF16, name="at")
                        nc.scalar.activation(at[:], sc[:], AF.Exp, scale=0.125)
                        ats[(tcc, hi2)] = at

                def av_mm(tcc):
                    for hi2 in range(2):
                        at = ats.pop((tcc, hi2))
                        h = 2 * pair + hi2
                        for hf in range(2):
                            nc.tensor.matmul(
                                avs[hi2][0 : DK + 1, hf * 512 : (hf + 1) * 512],
                                vh[:, tcc, h, :],
                                at[:, hf * 512 : (hf + 1) * 512],
                                start=(tcc == 0),
                                stop=(tcc == TC - 1),
                            )

                scores_exp(0)
                scores_exp(1)
                for tcc in range(2, TC):
                    scores_exp(tcc)
                    av_mm(tcc - 2)
                    drain_one()
                av_mm(TC - 2)
                av_mm(TC - 1)
                for n2 in pending[:]:
                    n2()
                pending.clear()

                # normalize part 1: copy unnormalized rows out of PSUM and
                # compute the reciprocal of the denominator row.
                norm_rs = []
                for hi2 in range(2):
                    av = avs[hi2]
                    rows = slice(DK * hi2, DK * (hi2 + 1))
                    u = xp.tile([P, ST], F32, name="xc")
                    nc.vector.tensor_copy(u[rows, :], av[0:DK, :])
                    dsb = xp.tile([1, ST], F32, name="xc")
                    nc.vector.tensor_copy(dsb[0:1, :], av[DK : DK + 1, :])
                    rsb = xp.tile([1, ST], F32, name="xc")
                    scr = xp.tile([1, ST], F32, name="xc")
                    nc.vector.reciprocal_approx_accurate(
                        rsb[0:1, :], dsb[0:1, :], scr[0:1, :]
                    )
                    rdr = dp.tile([1, ST], F32, name="rdr")
                    nc.sync.dma_start(rdr[0:1, :], rsb[0:1, :])
                    norm_rs.append((rdr, u))

                # normalize part 2 (deferred): broadcast the reciprocal via
                # DRAM round-trip, then fused multiply+cast to bf16.
                for hi2 in range(2):
                    rdr, u = norm_rs[hi2]
                    rows = slice(DK * hi2, DK * (hi2 + 1))
                    sl = (rows, slice(s0, s0 + ST))
                    rb = xp.tile([P, ST], F32, name="xc")

                    def t1(rdr=rdr, rb=rb, rows=rows):
                        nc.sync.dma_start(
                            rb[rows, :], rdr[0:1, :].to_broadcast((DK, ST))
                        )

                    def t2(u=u, rb=rb, rows=rows, sl=sl, o2h=o2h):
                        nc.vector.tensor_tensor(
                            out=o2h[sl], in0=u[rows, :], in1=rb[rows, :],
                            op=OP.mult,
                        )

                    pending.extend([t1, t2])

            # ---------------- issue order -----------------------------------
            proj_kq("k", bk_sb, kT, 0, (0, 1))
            proj_vh(0)
            proj_kq("q", bq_sb, qT, 0, (0,))
            attn_block(0, 0)
            proj_kq("q", bq_sb, qT, 0, (1,))
            attn_block(0, 1)
            proj_kq("k", bk_sb, kT, 1, (0, 1))
            proj_kq("q", bq_sb, qT, 1, (0, 1))
            proj_vh(1)
            attn_block(1, 0)
            attn_block(1, 1)

            # ---------------- output projection -----------------------------
            # st7 0..7 read s-columns < 1024, whose normalize (st2=0 blocks)
            # has already drained; emit them before the final norm2 drain so
            # the PE covers the last block's normalize latency.
            st7_order = list(range(TC // 2)) + [-1] + list(range(TC // 2, TC))
            for st7 in st7_order:
                if st7 == -1:
                    for n2 in pending[:]:
                        n2()
                    pending.clear()
                    continue
                if st7 < TC // 2:
                    drain_one()
                of_ps = ps_av.tile([P, ST], F32, name="av")
                for c in range(2):
                    o2h = (o2a, o2b)[c]
                    for nh in range(2):
                        nc.tensor.matmul(
                            of_ps[:, nh * 512 : (nh + 1) * 512],
                            o2h[:, st7 * P : (st7 + 1) * P],
                            wo_sb[:, c, nh * 512 : (nh + 1) * 512],
                            start=(c == 0),
                            stop=(c == 1),
                        )
                of = xp.tile([P, ST], BF16, name="xc")
                nc.vector.tensor_copy(of[:], of_ps[:])
                nc.scalar.dma_start(out[st7 * P : (st7 + 1) * P, :], of[:])

    nc.compile()
    return nc


def _get_nc():
    global _COMPILED
    if _COMPILED is None:
        _COMPILED = _build()
    return _COMPILED


def _bf16(x):
    import ml_dtypes

    return np.ascontiguousarray(x.astype(ml_dtypes.bfloat16))


def _make_in_maps(q, k, v, w_q, b_q, w_k, b_k, w_v, b_v, w_o, b_o):
    q = np.asarray(q, np.float32)
    k = np.asarray(k, np.float32)
    v = np.asarray(v, np.float32)
    xs = {}
    for t, arr in (("q", q), ("k", k), ("v", v)):
        for b in range(2):
            xs[(t, b)] = _bf16(np.ascontiguousarray(arr[b].T))
    ws = {"q": np.asarray(w_q, np.float32), "k": np.asarray(w_k, np.float32),
          "v": np.asarray(w_v, np.float32)}
    bs = {"q": np.asarray(b_q, np.float32), "k": np.asarray(b_k, np.float32),
          "v": np.asarray(b_v, np.float32)}
    w_o = np.asarray(w_o, np.float32)
    in_maps = []
    for core in range(N_CORES):
        b, hg = divmod(core, 4)
        sl = slice(hg * DH, (hg + 1) * DH)
        m = {}
        for t in ("q", "k", "v"):
            m[f"x{t}"] = xs[(t, b)]
            m[f"w{t}"] = _bf16(np.ascontiguousarray(ws[t][sl, :].T))
            m[f"b{t}"] = np.ascontiguousarray(bs[t][sl])
        m["wo"] = _bf16(np.ascontiguousarray(w_o[:, sl].T))
        in_maps.append(m)
    return in_maps


def run(inputs, trace=False):
    from concourse.bass_utils import run_bass_kernel_spmd

    nc = _get_nc()
    in_maps = _make_in_maps(**inputs)
    res = run_bass_kernel_spmd(
        nc, in_maps, core_ids=list(range(N_CORES)), trace=trace
    )
    b_o = np.asarray(inputs["b_o"], np.float32)
    full = np.empty((2, S, DM), np.float32)
    for b in range(2):
        acc = res.results[4 * b]["out"].astype(np.float32)
        for hg in range(1, 4):
            acc = acc + res.results[4 * b + hg]["out"].astype(np.float32)
        full[b] = acc + b_o[None, :]
    return full, res


def kernel(**inputs) -> np.ndarray:
    full, _ = run(inputs, trace=False)
    return full


# revision 8
# speedup vs baseline: 2.0512x; 1.1158x over previous
"""Multi-head attention (B=2, S=2048, D=1024, H=16, d_k=64) on 8 trn2 cores.

Sharding: batch (2) x head-groups (4 groups of 4 heads). Each core computes
its batch's full sequence for its 4 heads plus the partial output projection
(w_o row-sharded); host sums the 4 bf16 partials per batch and adds b_o.

Numerics: single-pass bf16 matmuls with fp32 PSUM accumulation everywhere
(the 2e-2 rel-err budget has ~10x headroom over bf16-grade ~2e-3 error).
exp() writes bf16 directly so the attention probabilities feed the AV
matmul with no intermediate cast.

Schedule: the exp stream on the scalar engine is the pacer (~1.11us per
[128,1024] tile, 128 tiles). Projections are issued per head-pair so the
first attention block starts as early as possible; the remaining
projections are issued in the inter-block gaps where the scalar engine
still has exp backlog. Input DMA is split across both HWDGE rings
(sync: x_k/x_q, scalar: weights + x_v + w_o).

Layout: all inputs host-pre-transposed to [d_model, seq]:
  qT/kT  = w_c @ x.T  -> [128(d' pair), 2, 2048] bf16
  vh     = x @ w_c.T  -> [128(t), tc, h, 65] bf16 (+ ones column)
  scoresT[t, s] (K=64, head pairs packed via tile_position row groups)
  atT    = exp(scoresT/8) bf16  (no max subtraction: scores ~ N(0,1))
  av     = [vh_h | 1].T @ atT   (fp32 psum, M=65: row 64 = denominators)
  out   += (av * bcast(1/denom)).T @ w_oT   (bf16 partial)
"""

import numpy as np

P = 128
S = 2048
DM = 1024
DH = 256          # head dims per core (4 heads x 64)
H = 4             # heads per core
DK = 64
MC = DM // P      # 8 m-chunks
TC = S // P       # 16 t-chunks
ST = 1024         # s-tile for scores/exp psum tiles
NST = S // ST     # 2
N_CORES = 8

_COMPILED = None


def _build():
    import concourse.bacc as bacc
    import concourse.mybir as mybir
    from concourse.tile import TileContext

    F32 = mybir.dt.float32
    BF16 = mybir.dt.bfloat16
    AF = mybir.ActivationFunctionType
    OP = mybir.AluOpType

    nc = bacc.Bacc(None, target_bir_lowering=False)

    xin = {}
    win = {}
    for t in ("q", "k", "v"):
        xin[t] = nc.dram_tensor(f"x{t}", [DM, S], BF16, kind="ExternalInput")
        win[t] = nc.dram_tensor(f"w{t}", [DM, DH], BF16, kind="ExternalInput")
    bq = nc.dram_tensor("bq", [DH], F32, kind="ExternalInput")
    bk = nc.dram_tensor("bk", [DH], F32, kind="ExternalInput")
    bv = nc.dram_tensor("bv", [DH], F32, kind="ExternalInput")
    wo = nc.dram_tensor("wo", [DH, DM], BF16, kind="ExternalInput")
    out = nc.dram_tensor("out", [S, DM], BF16, kind="ExternalOutput")

    with TileContext(nc) as tc:
        with (
            tc.tile_pool(name="persist", bufs=1) as pp,
            tc.tile_pool(name="xfull", bufs=24) as xw,
            tc.tile_pool(name="trans", bufs=10) as xp,
            tc.tile_pool(name="athl", bufs=8) as hp,
            tc.tile_pool(name="dram", bufs=4, space="DRAM") as dp,
            tc.tile_pool(name="ps_sc", bufs=2, space="PSUM") as ps_sc,
            tc.tile_pool(name="ps_av", bufs=2, space="PSUM") as ps_av,
        ):
            qT = pp.tile([P, 2, S], BF16, name="qT")
            kT = pp.tile([P, 2, S], BF16, name="kT")
            vh = pp.tile([P, TC, H, DK + 1], BF16, name="vh")
            wo_sb = pp.tile([P, 2, DM], BF16, name="wo_sb")
            o2a = pp.tile([P, S], BF16, name="o2a")  # heads 0,1 normalized
            o2b = pp.tile([P, S], BF16, name="o2b")  # heads 2,3
            bq_sb = pp.tile([P, 2], F32, name="bq_sb")
            bk_sb = pp.tile([P, 2], F32, name="bk_sb")
            bv_bc = pp.tile([P, DH], F32, name="bv_bc")

            nc.vector.memset(vh[:, :, :, DK : DK + 1], 1.0)
            nc.sync.dma_start(bq_sb[:], bq[:].rearrange("(c p) -> p c", p=P))
            nc.sync.dma_start(bk_sb[:], bk[:].rearrange("(c p) -> p c", p=P))
            nc.sync.dma_start(bv_bc[:], bv[None, :].to_broadcast((P, DH)))

            # ---------------- input DMA (two HWDGE rings) -------------------
            # sync ring: x_k then x_q (1 MB transfers). scalar ring: all
            # weights first (small), then x_v, then w_o. The scalar engine
            # only issues these before its first exp, so nothing competes
            # with the activation stream later.
            wt = {}
            for t in ("k", "q", "v"):
                w = pp.tile([P, MC, DH], BF16, name=f"w{t}_sb")
                nc.scalar.dma_start(
                    w[:], win[t][:].rearrange("(c p) n -> p c n", p=P)
                )
                wt[t] = w
            xt = {}
            for t in ("k", "q", "v"):
                eng = nc.scalar if t == "v" else nc.sync
                for mc in range(MC):
                    x = xw.tile([P, S], BF16, name="xc")
                    eng.dma_start(x[:], xin[t][mc * P : (mc + 1) * P, :])
                    xt[(t, mc)] = x
            nc.scalar.dma_start(wo_sb[:], wo[:].rearrange("(c p) n -> p c n", p=P))

            def xsl(t, mc):
                return xt[(t, mc)][:]

            # ---------------- projections, issued per pair ------------------
            def proj_kq(t, b_sb, dT, pair, st2s):
                tiles = {}
                for st2 in st2s:
                    pool = (ps_sc, ps_av)[st2]
                    tiles[st2] = pool.tile([P, ST], F32, name=("sc", "av")[st2])
                for mc in range(MC):
                    for st2 in st2s:
                        for hf in range(2):
                            nc.tensor.matmul(
                                tiles[st2][:, hf * 512 : (hf + 1) * 512],
                                wt[t][:, mc, pair * P : (pair + 1) * P],
                                xsl(t, mc)[
                                    :, st2 * ST + hf * 512 : st2 * ST + (hf + 1) * 512
                                ],
                                start=(mc == 0),
                                stop=(mc == MC - 1),
                            )
                for st2 in st2s:
                    sl = (slice(None), pair, slice(st2 * ST, (st2 + 1) * ST))
                    nc.vector.tensor_scalar(
                        out=dT[sl], in0=tiles[st2][:], scalar1=b_sb[:, pair : pair + 1],
                        scalar2=None, op0=OP.add,
                    )

            def proj_vh():
                for tcc in range(TC):
                    pool = (ps_sc, ps_av)[tcc % 2]
                    ps = pool.tile([P, DH], F32, name=("sc", "av")[tcc % 2])
                    for mc in range(MC):
                        nc.tensor.matmul(
                            ps[:],
                            xsl("v", mc)[:, tcc * P : (tcc + 1) * P],
                            wt["v"][:, mc, :],
                            start=(mc == 0),
                            stop=(mc == MC - 1),
                        )
                    nc.vector.tensor_tensor(
                        out=vh[:, tcc, :, 0:DK],
                        in0=ps[:].rearrange("p (h d) -> p h d", h=H),
                        in1=bv_bc[:].rearrange("p (h d) -> p h d", h=H),
                        op=OP.add,
                    )

            # ---------------- attention block -------------------------------
            pending = []  # deferred thunks drained one per tcc step

            def drain_one():
                if pending:
                    pending.pop(0)()

            def attn_block(pair, st2):
                o2h = (o2a, o2b)[pair]
                s0 = st2 * ST
                avs = [ps_av.tile([P, ST], F32, name="av") for _ in range(2)]
                ats = {}

                def scores_exp(tcc):
                    for hi2 in range(2):
                        rows = slice(DK * hi2, DK * (hi2 + 1))
                        sc = ps_sc.tile([P, ST], F32, name="sc")
                        for hf in range(2):
                            nc.tensor.matmul(
                                sc[:, hf * 512 : (hf + 1) * 512],
                                kT[rows, pair, tcc * P : (tcc + 1) * P],
                                qT[rows, pair, s0 + hf * 512 : s0 + (hf + 1) * 512],
                                start=True,
                                stop=True,
                                tile_position=(DK * hi2, 0),
                            )
                        at = hp.tile([P, ST], BF16, name="at")
                        nc.scalar.activation(at[:], sc[:], AF.Exp, scale=0.125)
                        ats[(tcc, hi2)] = at

                def av_mm(tcc):
                    for hi2 in range(2):
                        at = ats.pop((tcc, hi2))
                        h = 2 * pair + hi2
                        for hf in range(2):
                            nc.tensor.matmul(
                                avs[hi2][0 : DK + 1, hf * 512 : (hf + 1) * 512],
                                vh[:, tcc, h, :],
                                at[:, hf * 512 : (hf + 1) * 512],
                                start=(tcc == 0),
                                stop=(tcc == TC - 1),
                            )

                scores_exp(0)
                scores_exp(1)
                for tcc in range(2, TC):
                    scores_exp(tcc)
                    av_mm(tcc - 2)
                    drain_one()
                av_mm(TC - 2)
                av_mm(TC - 1)
                for n2 in pending[:]:
                    n2()
                pending.clear()

                # normalize part 1: copy unnormalized rows out of PSUM and
                # compute the reciprocal of the denominator row.
                norm_rs = []
                for hi2 in range(2):
                    av = avs[hi2]
                    rows = slice(DK * hi2, DK * (hi2 + 1))
                    u = xp.tile([P, ST], F32, name="xc")
                    nc.vector.tensor_copy(u[rows, :], av[0:DK, :])
                    dsb = xp.tile([1, ST], F32, name="xc")
                    nc.vector.tensor_copy(dsb[0:1, :], av[DK : DK + 1, :])
                    rsb = xp.tile([1, ST], F32, name="xc")
                    scr = xp.tile([1, ST], F32, name="xc")
                    nc.vector.reciprocal_approx_accurate(
                        rsb[0:1, :], dsb[0:1, :], scr[0:1, :]
                    )
                    rdr = dp.tile([1, ST], F32, name="rdr")
                    nc.sync.dma_start(rdr[0:1, :], rsb[0:1, :])
                    norm_rs.append((rdr, u))

                # normalize part 2 (deferred): broadcast the reciprocal via
                # DRAM round-trip, then fused multiply+cast to bf16.
                for hi2 in range(2):
                    rdr, u = norm_rs[hi2]
                    rows = slice(DK * hi2, DK * (hi2 + 1))
                    sl = (rows, slice(s0, s0 + ST))
                    rb = xp.tile([P, ST], F32, name="xc")

                    def t1(rdr=rdr, rb=rb, rows=rows):
                        nc.sync.dma_start(
                            rb[rows, :], rdr[0:1, :].to_broadcast((DK, ST))
                        )

                    def t2(u=u, rb=rb, rows=rows, sl=sl, o2h=o2h):
                        nc.vector.tensor_tensor(
                            out=o2h[sl], in0=u[rows, :], in1=rb[rows, :],
                            op=OP.mult,
                        )

                    pending.extend([t1, t2])

            # ---------------- issue order -----------------------------------
            # monolithic phase A: the attention blocks are exp-paced with
            # ~100% scalar-engine duty, so there is no backlog to hide
            # projections under — splitting them into the stream only adds
            # exposed gaps (and PSUM capacity forbids true interleaving).
            proj_kq("k", bk_sb, kT, 0, (0, 1))
            proj_kq("k", bk_sb, kT, 1, (0, 1))
            proj_kq("q", bq_sb, qT, 0, (0, 1))
            proj_kq("q", bq_sb, qT, 1, (0, 1))
            proj_vh()
            attn_block(0, 0)
            attn_block(0, 1)
            attn_block(1, 0)
            attn_block(1, 1)

            # ---------------- output projection -----------------------------
            # st7 0..7 read s-columns < 1024, whose normalize (st2=0 blocks)
            # has already drained; emit them before the final norm2 drain so
            # the PE covers the last block's normalize latency.
            st7_order = list(range(TC // 2)) + [-1] + list(range(TC // 2, TC))
            for st7 in st7_order:
                if st7 == -1:
                    for n2 in pending[:]:
                        n2()
                    pending.clear()
                    continue
                if st7 < TC // 2:
                    drain_one()
                of_ps = ps_av.tile([P, ST], F32, name="av")
                for c in range(2):
                    o2h = (o2a, o2b)[c]
                    for nh in range(2):
                        nc.tensor.matmul(
                            of_ps[:, nh * 512 : (nh + 1) * 512],
                            o2h[:, st7 * P : (st7 + 1) * P],
                            wo_sb[:, c, nh * 512 : (nh + 1) * 512],
                            start=(c == 0),
                            stop=(c == 1),
                        )
                of = xp.tile([P, ST], BF16, name="xc")
                nc.vector.tensor_copy(of[:], of_ps[:])
                nc.scalar.dma_start(out[st7 * P : (st7 + 1) * P, :], of[:])

    nc.compile()
    return nc


def _get_nc():
    global _COMPILED
    if _COMPILED is None:
        _COMPILED = _build()
    return _COMPILED


def _bf16(x):
    import ml_dtypes

    return np.ascontiguousarray(x.astype(ml_dtypes.bfloat16))


def _make_in_maps(q, k, v, w_q, b_q, w_k, b_k, w_v, b_v, w_o, b_o):
    q = np.asarray(q, np.float32)
    k = np.asarray(k, np.float32)
    v = np.asarray(v, np.float32)
    xs = {}
    for t, arr in (("q", q), ("k", k), ("v", v)):
        for b in range(2):
            xs[(t, b)] = _bf16(np.ascontiguousarray(arr[b].T))
    ws = {"q": np.asarray(w_q, np.float32), "k": np.asarray(w_k, np.float32),
          "v": np.asarray(w_v, np.float32)}
    bs = {"q": np.asarray(b_q, np.float32), "k": np.asarray(b_k, np.float32),
          "v": np.asarray(b_v, np.float32)}
    w_o = np.asarray(w_o, np.float32)
    in_maps = []
    for core in range(N_CORES):
        b, hg = divmod(core, 4)
        sl = slice(hg * DH, (hg + 1) * DH)
        m = {}
        for t in ("q", "k", "v"):
            m[f"x{t}"] = xs[(t, b)]
            m[f"w{t}"] = _bf16(np.ascontiguousarray(ws[t][sl, :].T))
            m[f"b{t}"] = np.ascontiguousarray(bs[t][sl])
        m["wo"] = _bf16(np.ascontiguousarray(w_o[:, sl].T))
        in_maps.append(m)
    return in_maps


def run(inputs, trace=False):
    from concourse.bass_utils import run_bass_kernel_spmd

    nc = _get_nc()
    in_maps = _make_in_maps(**inputs)
    res = run_bass_kernel_spmd(
        nc, in_maps, core_ids=list(range(N_CORES)), trace=trace
    )
    b_o = np.asarray(inputs["b_o"], np.float32)
    full = np.empty((2, S, DM), np.float32)
    for b in range(2):
        acc = res.results[4 * b]["out"].astype(np.float32)
        for hg in range(1, 4):
            acc = acc + res.results[4 * b + hg]["out"].astype(np.float32)
        full[b] = acc + b_o[None, :]
    return full, res


def kernel(**inputs) -> np.ndarray:
    full, _ = run(inputs, trace=False)
    return full
